# revision 31
# baseline (speedup 1.0000x reference)
"""AKT (monotonic attention with distance decay) Trainium2 kernel.

Strategy: data-parallel over batch (B=8) across 8 NeuronCores; each core
computes one batch element's full transformer-attention layer:
  qh/kh/vh projections -> scores -> masked softmax -> suffix-sum distance
  decay -> decayed softmax -> attn @ vh -> output projection.

Layout choices (per core):
  * host pre-transposes q/k/v to [D, S] fp16 so every x @ W matmul has its
    contraction on the partition axis without on-chip transposes.
  * score strips are [128 queries, N keys] (N = (qb+1)*128, causal lower
    triangle only, packed per head into a [128, PACKW] row).  Masking is
    done by accumulating a -60000 lower-tri constant tile into PSUM with
    one extra matmul per strip.
  * E = exp(scores) per strip on ScalarE; suffix sums U via reversed
    tensor_tensor_scan on VectorE; Z recovered as U[:,0] + E[:,0] (scan
    byproduct — no accum_out reads on the E pass).
  * sqrt(U) and sqrt(Z/g^2) run in a per-head-pair Sqrt-table window (2
    LoadActFuncSet per pair); the decay arg m' = (sqrt(U)*g*rsqrt(Z)) *
    (-sqrt|i-t|) is fused on VectorE via scalar_tensor_tensor; w = exp(m')
    and su = sqrt(U) are emitted as HALF-width activations so downstream
    strips start before the whole head finishes (barrier splitting).
  * F = exp(scores*w) per strip (accum_out gives z2), normalized in place,
    PE-transposed per 128-block and fed to attn @ vh with vh stationary;
    output projection maps outT back to [S, D] with Wo.
  * Cross-rep overlap: q/k projections are parity-tagged and emitted
    mid-rep for the NEXT rep, and vh/y PSUM shares the ot slot, so the
    next rep's first score strips start during this rep's output tail
    (removed a ~21us/rep ACT bubble at the rep boundary).
  * scan initial=1e-6 floors U (and hence Z) away from 0, replacing the
    explicit clamp; sqz is emitted before the wide su sqrts so the DVE
    recip+stt chain overlaps the sqrt-table window.
  * Engine balance (CoreSim, per rep): DVE ~186us, ACT ~186us, PE ~90us,
    steady span ~205us; HW 232.7us/rep (was 277.6 baseline).
    Buffer-depth retunes (e bufs 2 + g bufs 4) looked −3us/rep in sim
    but measured +15us on HW — schedule-sensitive sim deltas must be
    HW-verified before keeping.
    Dead ends (HW-verified): GPSIMD/Pool per-op overhead is ~us-scale
    (normalize on Pool = 2.8x regression); walrus rejects scan/stt
    opcodes on Pool and any Pool PSUM access; TRN2 matmul cannot write
    f16 PSUM (so the g-mult TT stays 1x against f32 scores PSUM);
    transpose rhs must be a permutation (no diag(rz2) fold).
"""

import os
import sys

for _p in ("/opt/trn_rl_repo",):
    if os.path.isdir(_p) and _p not in sys.path:
        sys.path.insert(0, _p)

import numpy as np

import concourse.bass as bass
import concourse.mybir as mybir
import concourse.tile as tile
from concourse import bacc
from concourse.bass_utils import run_bass_kernel_spmd
from concourse.masks import make_identity, make_lower_triangular

# The act-table insertion pass assigns each activation the FIRST table set
# containing its function, so Exp (set 0) and Ln (set 5) ping-pong ~80 table
# loads (~2.7us each). All functions this kernel uses live together in
# natural_log_exp_and_others; strip them from the earlier sets so every
# activation resolves to that one set (set ids keep their original positions,
# so walrus still loads the right tables).
_orig_gat = bacc.get_activation_tables


def _patched_gat(arch):
    tabs = _orig_gat(arch)
    target = None
    for name, fns in tabs.items():
        names = {f.name for f in fns}
        if "Exp" in names and "Ln" in names:
            target = name
            break
    if target is None:
        return tabs
    covered = tabs[target]
    out = {}
    seen_target = False
    for name, fns in tabs.items():
        if name == target:
            seen_target = True
            out[name] = fns
        elif not seen_target:
            out[name] = fns - covered
        else:
            out[name] = fns
    return out


bacc.get_activation_tables = _patched_gat

B, S, D, H = 8, 1024, 256, 8
DK = D // H
NQ = S // 128          # number of 128-row query strips
SCALE = 1.0 / np.sqrt(DK)
MASKVAL = -60000.0     # representable in fp16; exp(-60000) == 0 in fp32
F32 = mybir.dt.float32
F16 = mybir.dt.float16
BF16 = mybir.dt.bfloat16
AX = mybir.AluOpType
ACTF = mybir.ActivationFunctionType

PD_BASE = 1024         # master tile: PDm[p, u] = max(p + u - PD_BASE, 1)
PD_W = 2048
OFF = [128 * qb * (qb + 1) // 2 for qb in range(NQ + 1)]  # packed strip offsets
PACKW = OFF[NQ]


def _build_program(g2_per_head, reps=1):
    """Build the single-core Bass program (identical on all 8 cores)."""
    nc = bacc.Bacc("TRN2", target_bir_lowering=False, debug=False, num_devices=8)

    # ---- DRAM parameters (per-core shards) ----
    qT = nc.dram_tensor("qT", [D, S], F16, kind="ExternalInput").ap()
    kT = nc.dram_tensor("kT", [D, S], F16, kind="ExternalInput").ap()
    vT = nc.dram_tensor("vT", [D, S], F16, kind="ExternalInput").ap()
    WqT = nc.dram_tensor("WqT", [D, D], F16, kind="ExternalInput").ap()
    WkT = nc.dram_tensor("WkT", [D, D], F16, kind="ExternalInput").ap()
    WvT = nc.dram_tensor("WvT", [D, D], F16, kind="ExternalInput").ap()
    WoT = nc.dram_tensor("WoT", [D, D], F32, kind="ExternalInput").ap()
    bqs = nc.dram_tensor("bqs", [D, 1], F32, kind="ExternalInput").ap()  # bq*SCALE
    bk_c = nc.dram_tensor("bk_c", [D, 1], F32, kind="ExternalInput").ap()
    bv_r = nc.dram_tensor("bv_r", [1, D], F16, kind="ExternalInput").ap()
    bo_r = nc.dram_tensor("bo_r", [1, D], F32, kind="ExternalInput").ap()
    sqPD = nc.dram_tensor("sqPD", [128, PD_W], F16, kind="ExternalInput").ap()
    out = nc.dram_tensor("out", [S, D], F32, kind="ExternalOutput").ap()

    with tile.TileContext(nc) as tc:
        _body(tc, qT, kT, vT, WqT, WkT, WvT, WoT, bqs, bk_c, bv_r, bo_r,
              sqPD, out, g2_per_head, reps=reps)
    nc.compile()
    return nc


def _body(tc, qT, kT, vT, WqT, WkT, WvT, WoT, bqs, bk_c, bv_r, bo_r,
          sqPD, out, g2_per_head, reps=1):
    nc = tc.nc
    from contextlib import ExitStack
    ctx = ExitStack()

    const_pool = ctx.enter_context(tc.tile_pool(name="const", bufs=1))
    in_pool = ctx.enter_context(tc.tile_pool(name="inp", bufs=1))
    proj_pool = ctx.enter_context(tc.tile_pool(name="proj", bufs=1))
    ppsum = ctx.enter_context(tc.tile_pool(name="ppsum", bufs=2, space="PSUM"))
    spsum = ppsum
    ftpsum = ctx.enter_context(tc.tile_pool(name="ftpsum", bufs=2, space="PSUM"))
    otpsum = ctx.enter_context(tc.tile_pool(name="otpsum", bufs=1, space="PSUM"))
    e_pool = ctx.enter_context(tc.tile_pool(name="epool", bufs=3))
    u_pool = ctx.enter_context(tc.tile_pool(name="upool", bufs=3))
    su_pool = ctx.enter_context(tc.tile_pool(name="supool", bufs=3))
    wk_pool = ctx.enter_context(tc.tile_pool(name="wkpool", bufs=3))
    fk_pool = ctx.enter_context(tc.tile_pool(name="fkpool", bufs=3))
    g_pool = ctx.enter_context(tc.tile_pool(name="gpool", bufs=3))
    ft_pool = ctx.enter_context(tc.tile_pool(name="ftpool", bufs=3))
    z_pool = ctx.enter_context(tc.tile_pool(name="zpool", bufs=1))
    y_pool = ctx.enter_context(tc.tile_pool(name="ypool", bufs=2))

    # ---- constants / masters ----
    I128 = const_pool.tile([128, 128], F16)
    make_identity(nc, I128[:])
    MASKT = const_pool.tile([128, 128], F16)
    make_lower_triangular(nc, MASKT[:], val=MASKVAL, diag=True)
    ONES1 = const_pool.tile([1, 128], F16)
    nc.vector.memset(ONES1[:], 1.0)
    ZEROSB = const_pool.tile([128, 1024], BF16)
    nc.vector.memset(ZEROSB[:], 0.0)


    # ---- load inputs ----
    def load2(dram, nm, dtype=F16, w=S):
        ts = []
        for cc in range(2):
            t = in_pool.tile([128, w], dtype, tag=f"{nm}{cc}", name=f"{nm}{cc}")
            nc.sync.dma_start(t[:], dram[cc * 128:(cc + 1) * 128, :])
            ts.append(t)
        return ts

    qT_sb = load2(qT, "qT")
    kT_sb = load2(kT, "kT")
    vT_sb = load2(vT, "vT")
    WqT_sb = load2(WqT, "WqT", w=D)
    WkT_sb = load2(WkT, "WkT", w=D)
    WvT_sb = load2(WvT, "WvT", w=D)
    WoT_sb = load2(WoT, "WoT", dtype=F32, w=D)
    bqs_sb = load2(bqs, "bqs", dtype=F32, w=1)
    bk_sb = load2(bk_c, "bkc", dtype=F32, w=1)
    bv_sb = in_pool.tile([1, D], F16)
    nc.sync.dma_start(bv_sb[:], bv_r[:, :])
    bo_sb = in_pool.tile([1, D], F32)
    nc.sync.dma_start(bo_sb[:], bo_r[:, :])
    ONES1F = const_pool.tile([1, 128], F32)
    nc.vector.memset(ONES1F[:], 1.0)
    sqPD_sb = in_pool.tile([128, PD_W], F16, tag="sqPD", name="sqPD_sb")
    nc.sync.dma_start(sqPD_sb[:], sqPD[:, :])

    # ---- projections: qhT/khT [d', i] (2 chunks of 128), vh [t, d'] (8 blocks)
    # parity-tagged so rep r+1's projections overlap rep r's tail instead of
    # serializing the rep boundary (the ~21us/rep ACT bubble)
    def emit_proj(par):
        qhT, khT = [], []
        for which, (W_sb, x_sb, dst_list, scale, bias_sb) in enumerate((
                (WqT_sb, qT_sb, qhT, SCALE, bqs_sb),
                (WkT_sb, kT_sb, khT, 1.0, bk_sb))):
            for cc in range(2):
                ps = ppsum.tile([128, S], F32, tag="ps32", name="projps")
                for half in range(2):
                    c0, c1 = half * 512, (half + 1) * 512
                    for ec in range(2):
                        nc.tensor.matmul(
                            ps[:, c0:c1],
                            lhsT=W_sb[ec][:, cc * 128:(cc + 1) * 128],
                            rhs=x_sb[ec][:, c0:c1],
                            start=(ec == 0), stop=(ec == 1))
                dst = proj_pool.tile([128, S], F16,
                                     tag=f"proj{which}{cc}p{par}",
                                     name=f"proj{which}{cc}p{par}")
                nc.scalar.activation(dst[:], ps[:], ACTF.Identity,
                                     bias=bias_sb[cc][:, 0:1], scale=scale)
                dst_list.append(dst)

        vh = []
        for tb in range(NQ):
            # vh/y share the otpsum slot, keeping the ps32 slots free for
            # the next rep's first score strips at the rep boundary
            ps = otpsum.tile([128, D], F32, tag="ot", name="vps")
            for ec in range(2):
                nc.tensor.matmul(ps[:],
                                 lhsT=vT_sb[ec][:, tb * 128:(tb + 1) * 128],
                                 rhs=WvT_sb[ec][:], start=(ec == 0),
                                 stop=False)
            nc.tensor.matmul(ps[:], lhsT=ONES1[:], rhs=bv_sb[:],
                             start=False, stop=True)
            dst = proj_pool.tile([128, D], F16, tag=f"vh{tb}p{par}",
                                 name=f"vh{tb}p{par}")
            nc.scalar.activation(dst[:], ps[:], ACTF.Copy)
            vh.append(dst)
        return qhT, khT, vh

    # per-head output accumulation target [hd, i] fp32, 2 chunks
    outTn_sb = [proj_pool.tile([128, S], F32, tag=f"otn{c}", name=f"otn{c}")
                for c in range(2)]

    def score_psum(h, qb, N, qhT_sb, khT_sb):
        """scores strip [128, N] for head h, q-block qb, causal-masked."""
        hc, hm = h // 4, h % 4
        ps = spsum.tile([128, 1024], F32, tag="ps32", name="sps")[:, 0:N]
        lhs = qhT_sb[hc][32 * hm:32 * hm + 32, qb * 128:qb * 128 + 128]
        kh = khT_sb[hc]
        c = 0
        while c < qb * 128:      # full blocks strictly left of the diagonal
            c2 = min(c + 512, qb * 128)
            nc.tensor.matmul(ps[:, c:c2], lhsT=lhs,
                             rhs=kh[32 * hm:32 * hm + 32, c:c2],
                             start=True, stop=True,
                             tile_position=(32 * hm, 0))
            c = c2
        # diagonal block: scores then causal mask accumulate
        nc.tensor.matmul(ps[:, qb * 128:N], lhsT=lhs,
                         rhs=kh[32 * hm:32 * hm + 32, qb * 128:N],
                         start=True, stop=False,
                         tile_position=(32 * hm, 0))
        nc.tensor.matmul(ps[:, qb * 128:N], lhsT=MASKT[:], rhs=I128[:],
                         start=False, stop=True)
        return ps

    projs = {0: emit_proj(0)}

    for _rep in range(reps):
        par = _rep % 2
        qhT_sb, khT_sb, vh_sb = projs.pop(par)
        pstate = {}

        def phase1(pair):
            heads = (2 * pair, 2 * pair + 1)
            z_all = {h: z_pool.tile([128, NQ], F32,
                                    tag=f"z{pair%2}{h%2}", name=f"z{h}")
                     for h in heads}
            e_tiles = {h: e_pool.tile([128, PACKW], BF16, tag="e",
                                      name=f"e{h%2}") for h in heads}
            u_tiles = {h: u_pool.tile([128, PACKW], BF16, tag="ub",
                                      name=f"ub{h%2}") for h in heads}
            pstate[pair] = (heads, z_all, u_tiles)
            # strip-level interleave of the head pair: consecutive score
            # matmuls land in different PE row groups (tile_position 32*hm)
            # (packed PACKW e tiles double as backpressure: per-strip e
            # tiles let the next pair's E exps flood the sqrt-table window,
            # 8 -> 11+ LoadActFuncSet/rep)
            for qb in range(NQ):
                N = (qb + 1) * 128
                o = OFF[qb]
                for h in heads:
                    ub = u_tiles[h]
                    e_pk = e_tiles[h]
                    ps = score_psum(h, qb, N, qhT_sb, khT_sb)
                    nc.scalar.activation(e_pk[:, o:o + N], ps[:, 0:N],
                                         ACTF.Exp)
                    # (walrus rejects the scan opcode on Pool, so it stays
                    # on DVE at 1x — it is an inherently serial recurrence)
                    nc.vector.memset(ub[:, o + N - 1:o + N], 0.0)
                    # initial=1e-6 floors U away from 0, replacing the
                    # explicit max(z, 1e-6) clamp before sqz
                    nc.vector.tensor_tensor_scan(
                        out=ub[:, o + N - 2:o - 1 if o else None:-1],
                        data0=e_pk[:, o + N - 1:o:-1],
                        data1=ZEROSB[:, 0:N - 1],
                        initial=1e-6, op0=AX.add, op1=AX.add)
                    # Z = full row sum = suffix-at-0 + E[:,0] (scan byproduct)
                    # (HW GPSIMD per-op overhead is ~us-scale — keep on DVE)
                    nc.vector.tensor_tensor(z_all[h][:, qb:qb + 1],
                                            ub[:, o:o + 1], e_pk[:, o:o + 1],
                                            op=AX.add)

        def phase23a(pair):
            heads, z_all, u_tiles = pstate[pair]
            # sqrt-table window: su = sqrt(U) full-width + sqz = sqrt(Z/g^2)
            su_map, sq_map = {}, {}
            # sqz first: the tiny sqrt unblocks the DVE recip+stt chain,
            # which then overlaps with the wide su sqrts on ACT
            for h in heads:
                sqz = z_pool.tile([128, NQ], F32, tag=f"sqz{h%2}")
                nc.scalar.activation(sqz[:], z_all[h][:], ACTF.Sqrt,
                                     scale=float(1.0 / g2_per_head[h]))
                sq_map[h] = sqz
            for h in heads:
                su_t = su_pool.tile([128, PACKW], BF16, tag="su",
                                    name=f"su{h%2}")
                # two half-width ops: strips 0-5 usable before 6-7 finish
                nc.scalar.activation(su_t[:, 0:OFF[6]],
                                     u_tiles[h][:, 0:OFF[6]], ACTF.Sqrt)
                nc.scalar.activation(su_t[:, OFF[6]:PACKW],
                                     u_tiles[h][:, OFF[6]:PACKW], ACTF.Sqrt)
                su_map[h] = su_t
            pstate[pair] = (heads, z_all, u_tiles, su_map, sq_map)

        def phase23b(pair):
            heads, z_all, u_tiles, su_map, sq_map = pstate.pop(pair)
            for h in heads:
                ub = u_tiles[h]
                su_t = su_map[h]
                # s = g * rsqrt(Z) = 1 / sqrt(Z/g^2)
                s_all = z_pool.tile([128, NQ], F32, tag=f"sall_{h%2}")
                nc.vector.reciprocal(s_all[:], sq_map[h][:])
                # m' = (su * s) * (-sqrt|i-t|), overwriting U (f16 view)
                for qb in range(NQ):
                    N = (qb + 1) * 128
                    o = OFF[qb]
                    u0 = PD_BASE + qb * 128
                    m_ap = ub[:, o:o + N].bitcast(F16)
                    nc.vector.scalar_tensor_tensor(
                        m_ap, su_t[:, o:o + N], s_all[:, qb:qb + 1],
                        sqPD_sb[:, u0:u0 - N:-1], op0=AX.mult, op1=AX.mult)
                # w = exp(m') in two half-width activations (earlier
                # strip availability for the G-mult chain)
                w_pk = wk_pool.tile([128, PACKW], F16, tag="w", name="w_pk")
                nc.scalar.activation(w_pk[:, 0:OFF[6]],
                                     ub[:, 0:OFF[6]].bitcast(F16), ACTF.Exp)
                nc.scalar.activation(w_pk[:, OFF[6]:PACKW],
                                     ub[:, OFF[6]:PACKW].bitcast(F16),
                                     ACTF.Exp)
                # F = exp(scores * w) per strip, row sums via accum
                f_pk = fk_pool.tile([128, PACKW], F16, tag="f", name="f_pk")
                z2 = z_pool.tile([128, NQ], F32, tag=f"z2_{h%2}",
                                 name=f"z2_{h%2}")
                for qb in range(NQ):
                    N = (qb + 1) * 128
                    o = OFF[qb]
                    ps = score_psum(h, qb, N, qhT_sb, khT_sb)
                    g_t = g_pool.tile([128, 1024], F16)
                    nc.vector.tensor_tensor(g_t[:, 0:N], ps[:, 0:N],
                                            w_pk[:, o:o + N], op=AX.mult)
                    nc.scalar.activation(f_pk[:, o:o + N], g_t[:, 0:N],
                                         ACTF.Exp,
                                         accum_out=z2[:, qb:qb + 1])
                z2m = z_pool.tile([128, NQ], F32, tag="z2m")
                nc.vector.tensor_scalar_max(z2m[:], z2[:], 1e-6)
                rz2 = z_pool.tile([128, NQ], F32, tag="rz2")
                nc.vector.reciprocal(rz2[:], z2m[:])
                # normalize F in place (packed, per-strip row scale)
                # (HW GPSIMD per-op overhead is ~us-scale — keep on DVE)
                for qb in range(NQ):
                    N = (qb + 1) * 128
                    o = OFF[qb]
                    nc.vector.tensor_scalar(
                        f_pk[:, o:o + N], f_pk[:, o:o + N],
                        rz2[:, qb:qb + 1], None, op0=AX.mult)

                ot_ps = otpsum.tile([32, S], F32, tag="ot", name="ot_ps")
                for tb in range(NQ):
                    width = (NQ - tb) * 128
                    ft_ps = ftpsum.tile([128, 1024], F16)
                    for qb in range(tb, NQ):
                        nc.tensor.transpose(
                            ft_ps[:, (qb - tb) * 128:(qb - tb + 1) * 128],
                            f_pk[:, OFF[qb] + tb * 128:OFF[qb] + (tb + 1) * 128],
                            I128[:])
                    ft_sb = ft_pool.tile([128, 1024], F16)
                    nc.vector.tensor_copy(ft_sb[:, 0:width], ft_ps[:, 0:width])
                    # accumulate outT[d, q] += vh[t, d]^T . F_hat^T[t, q]
                    for half in range(2):
                        c0 = max(half * 512, tb * 128)
                        c1 = (half + 1) * 512
                        if c0 >= c1:
                            continue
                        tb_max = min(NQ - 1, (c1 - 1) // 128)
                        nc.tensor.matmul(
                            ot_ps[0:32, c0:c1],
                            lhsT=vh_sb[tb][:, h * 32:(h + 1) * 32],
                            rhs=ft_sb[:, c0 - tb * 128:c1 - tb * 128],
                            start=(tb == 0), stop=(tb == tb_max))
                nc.vector.tensor_copy(
                    outTn_sb[h // 4][32 * (h % 4):32 * (h % 4) + 32, :],
                    ot_ps[0:32, :])

        # sequential driver: the tile scheduler already overlaps pairs;
        # grouping pairs to share sqrt-table windows backfires (the
        # scheduler interleaves exp ops into the window: 8 -> 15 loads).
        # The next rep's projections are emitted mid-rep (parity-tagged
        # tiles) so its first score strips need not wait at the boundary.
        for pair in range(4):
            phase1(pair)
            phase23a(pair)
            phase23b(pair)
            if pair == 1 and _rep + 1 < reps:
                projs[1 - par] = emit_proj(1 - par)

        # ---- output projection: Y[i, e] = outTn^T . WoT + bo
        for ib in range(NQ):
            y_ps = otpsum.tile([128, D], F32, tag="ot", name="yps")
            for hc in range(2):
                nc.tensor.matmul(y_ps[:], lhsT=outTn_sb[hc][:, ib * 128:(ib + 1) * 128],
                                 rhs=WoT_sb[hc][:], start=(hc == 0), stop=False)
            nc.tensor.matmul(y_ps[:], lhsT=ONES1F[:], rhs=bo_sb[:],
                             start=False, stop=True)
            y_sb = y_pool.tile([128, D], F32)
            nc.vector.tensor_copy(y_sb[:], y_ps[:])
            nc.sync.dma_start(out[ib * 128:(ib + 1) * 128, :], y_sb[:])


    ctx.close()


_PROGRAM_CACHE = {}


def _get_program(g2_key):
    if g2_key not in _PROGRAM_CACHE:
        _PROGRAM_CACHE[g2_key] = _build_program(np.array(g2_key))
    return _PROGRAM_CACHE[g2_key]


def _make_in_maps(q, k, v, Wq, bq, Wk, bk, Wv, bv, Wo, bo):
    pp, uu = np.meshgrid(np.arange(128), np.arange(PD_W), indexing="ij")
    sq_master = (-np.sqrt(np.maximum(pp + uu - PD_BASE, 1.0))).astype(np.float16)
    common = {
        "sqPD": sq_master,
        "WqT": np.ascontiguousarray(Wq.T).astype(np.float16),
        "WkT": np.ascontiguousarray(Wk.T).astype(np.float16),
        "WvT": np.ascontiguousarray(Wv.T).astype(np.float16),
        "WoT": np.ascontiguousarray(Wo.T).astype(np.float32),
        "bqs": (bq * SCALE).astype(np.float32).reshape(D, 1),
        "bk_c": bk.astype(np.float32).reshape(D, 1),
        "bv_r": bv.astype(np.float16).reshape(1, D),
        "bo_r": bo.astype(np.float32).reshape(1, D),
    }
    in_maps = []
    for b in range(B):
        m = dict(common)
        m["qT"] = np.ascontiguousarray(q[b].T).astype(np.float16)
        m["kT"] = np.ascontiguousarray(k[b].T).astype(np.float16)
        m["vT"] = np.ascontiguousarray(v[b].T).astype(np.float16)
        in_maps.append(m)
    return in_maps


def kernel(q, k, v, mask, Wq, bq, Wk, bk, Wv, bv, Wo, bo, gammas, zero_pad,
           **_unused):
    q = np.asarray(q, np.float32)
    k = np.asarray(k, np.float32)
    v = np.asarray(v, np.float32)
    gam = np.asarray(gammas, np.float32).reshape(H)
    g_abs = np.log1p(np.exp(gam))          # softplus(gammas) = |gamma| used in decay
    g2 = tuple((g_abs.astype(np.float64) ** 2).tolist())
    assert int(np.asarray(zero_pad)) == 1, "kernel specialized for zero_pad=1"

    nc = _get_program(g2)
    in_maps = _make_in_maps(np.asarray(q), np.asarray(k), np.asarray(v),
                            np.asarray(Wq), np.asarray(bq), np.asarray(Wk),
                            np.asarray(bk), np.asarray(Wv), np.asarray(bv),
                            np.asarray(Wo), np.asarray(bo))
    res = run_bass_kernel_spmd(nc, in_maps, core_ids=list(range(B)))
    outs = [np.asarray(res.results[b]["out"], np.float32) for b in range(B)]
    return np.stack(outs, axis=0)


if __name__ == "__main__":
    # quick single-core CoreSim check against a numpy reference
    from concourse.bass_interp import CoreSim

    rng = np.random.default_rng(0)
    q = rng.standard_normal((B, S, D), np.float32)
    k = rng.standard_normal((B, S, D), np.float32)
    v = rng.standard_normal((B, S, D), np.float32)
    sc = 1.0 / np.sqrt(D)
    Wq = rng.standard_normal((D, D), np.float32) * sc
    Wk = rng.standard_normal((D, D), np.float32) * sc
    Wv = rng.standard_normal((D, D), np.float32) * sc
    Wo = rng.standard_normal((D, D), np.float32) * sc
    bq = bk = bv = bo = np.zeros(D, np.float32)
    gammas = rng.standard_normal((H, 1, 1), np.float32) * 0.5

    def ref_one(b):
        def heads(x, W, bias):
            return (x @ W.T + bias).reshape(S, H, DK).transpose(1, 0, 2)
        qh, kh, vh = heads(q[b], Wq, bq), heads(k[b], Wk, bk), heads(v[b], Wv, bv)
        scores = np.einsum("hsd,htd->hst", qh, kh) / np.sqrt(DK)
        m = np.tril(np.ones((S, S), bool), k=-1)[None]
        x = np.where(m, scores, -1e30)
        x = x - x.max(-1, keepdims=True)
        smx = np.exp(x)
        smx = smx / smx.sum(-1, keepdims=True) * m
        distcum = np.cumsum(smx, -1)
        disttot = smx.sum(-1, keepdims=True)
        idx = np.arange(S, dtype=np.float32)
        pe = np.abs(idx[None, :] - idx[:, None])
        ds = np.sqrt(np.clip((disttot - distcum) * pe, 0, None))
        gamma = -np.log1p(np.exp(gammas.reshape(H, 1, 1)))
        te = np.clip(np.exp(ds * gamma), 1e-5, 1e5)
        sc2 = np.where(m, scores * te, -1e30)
        sc2 = sc2 - sc2.max(-1, keepdims=True)
        a = np.exp(sc2)
        a = a / a.sum(-1, keepdims=True)
        a[:, 0, :] = 0.0
        o = np.einsum("hst,htd->hsd", a, vh)
        return o.transpose(1, 0, 2).reshape(S, D) @ Wo.T + bo

    g_abs = np.log1p(np.exp(gammas.reshape(H)))
    nc = _build_program(g_abs ** 2)
    in_maps = _make_in_maps(q, k, v, Wq, bq, Wk, bk, Wv, bv, Wo, bo)

    sim = CoreSim(nc)
    for name, val in in_maps[0].items():
        sim.tensor(name)[:] = val
    sim.simulate()
    got = np.asarray(sim.tensor("out"), np.float32)
    want = ref_one(0)
    err = np.abs(got - want)
    rel = np.linalg.norm(got - want) / np.linalg.norm(want)
    print("max abs err:", err.max(), " rel l2:", rel)
    bad = np.unravel_index(err.argmax(), err.shape)
    print("worst at", bad, got[bad], want[bad])

